# revision 31
# baseline (speedup 1.0000x reference)
"""Multi-head attention Bass/Tile kernel for TRN2, 8-core SPMD.

Sharding: core c handles batch b = c//2, query-half qh = c%2. The host
rotates the token axis per core so query rows sit at [0:TQ] (attention is
key-permutation invariant), and gathers the unmasked keys (mask compaction)
so K/V projection + attention only touch TK <= T key tokens.

Final architecture (378us baseline -> ~226us):
  - hp-major single pass: for each head-pair, attention over both query
    halves, with Q/K/V projection chains for later head-pairs popped as
    "filler" PE work inside the attention loop.
  - attention inner loop is SOFTWARE PIPELINED: S-pairs run two j-steps
    ahead of their exp, PV/den consume e one exp behind, so PE and ACT
    never block each other in steady state (exp cadence = back-to-back
    ~1.0us; PE paces at ~1.2us/j-step).
  - all four projections (Q/K/V/O) run fp8 e4m3 with DoubleRow: each
    matmul contracts a PAIR of 128-chunks, halving projection PE time.
    wq is sent UNSCALED (wq/8 is fp8-subnormal); 1/sqrt(DH) rides the
    Exp activation's free affine scale. rel err ~1e-3 (vs 6.6e-5 bf16).
  - reciprocal_approx_fast for the softmax denominator, reading pvP/den
    straight from PSUM (no evacuation copy).
  - LayerNorm: mean rides the residual-add stt accum_out; var/normalize
    run on ACT (Square+accum, Identity w/ per-partition scale+bias) for
    the tail tiles but on DVE (stt-square, 2-scalar tensor_scalar) for
    the tiles that execute under the last attend, keeping ACT free for
    exp there; the g/b multiply-add is skipped entirely when the host
    detects ln_g==1, ln_b==0 (general path kept otherwise).
  - epilogues: first-half tiles run under the last head-pair's second
    attention pass; the 4 tail tiles are emitted STAGE-MAJOR so their
    chains pipeline across the strict per-engine FIFOs.
  - DMA: inputs host-swizzled to partition-major [128, chunk, cols] and
    split head/rest, one dma_start each, spread over the Sync and GpSimd
    DGE queues in consumption order, hTk first (each dma_start costs
    ~650ns of issuing-engine time, and early per-queue bandwidth -- not
    just bytes -- sets the pipeline-fill latency).
PSUM: 2x score tiles (4 banks) + pv|den tile (2) + proj/o-proj ring (2).

Matmul layouts (out = lhsT.T @ rhs, contraction on partitions):
  QT/KT [F, *] bf16 : lhsT=w*T [D,F] chunks, rhs=hT* [D,*] chunks
  V     [TK, F] bf16: lhsT=hTk chunk [D, t128], rhs=wvT [D, F]
  S^T   [j, (h0 i512 | h1 i512)] psum 2 banks, row-split tile_position
  exp   one ACT op per j-tile: [128, 1024] psum->sbuf bf16, bias=maskbias
  PV+den one psum tile [128, (pv 512 | den 512)]: V cols / ones lhsT
  O     [t, D] psum : lhsT=AVT [f, t128], rhs=woT [f, D] in 512 chunks
"""
import numpy as np
import ml_dtypes

import concourse.bass as bass
import concourse.tile as tile
from concourse import bacc, mybir

F32 = mybir.dt.float32
BF16 = mybir.dt.bfloat16
FP8 = mybir.dt.float8e4
DR = mybir.MatmulPerfMode.DoubleRow
AF = mybir.ActivationFunctionType
ALU = mybir.AluOpType

NEG_BIG = -1.0e30


def _pin_act_tables():
    """Force every ACT func we use (Exp, Ln, Square, Identity, Copy) to
    resolve to the single `natural_log_exp_and_others` table set, so the
    kernel does exactly one ACT_TABLE_LOAD instead of thrashing (~2.6us
    per switch). Preserves dict order (set ids are positional)."""
    import concourse.hw_specs as hw_specs
    if getattr(hw_specs, "_mha_tables_pinned", False):
        return
    orig = hw_specs.get_activation_tables

    def patched(module_arch):
        tabs = orig(module_arch)
        pin = "natural_log_exp_and_others"
        if pin in tabs:
            pinned_funcs = tabs[pin]
            for name, fns in tabs.items():
                if name != pin:
                    tabs[name] = fns - pinned_funcs
        return tabs

    hw_specs.get_activation_tables = patched
    import concourse.bacc as bacc_mod
    bacc_mod.get_activation_tables = patched
    hw_specs._mha_tables_pinned = True


def _chunks(total, step):
    out = []
    off = 0
    while off < total:
        out.append((off, min(step, total - off)))
        off += step
    return out


def build_nc(T, TQ, TK, D, NH, DH, n_cores=8, debug=False, triv_gb=False):
    """Build the single-core SPMD Bass program. TK = compacted key count."""
    F = NH * DH
    DC = D // 128        # D contraction chunks
    FC = F // 128        # feature chunks (2 heads per chunk, DH=64)
    KC = TK // 128       # key tiles
    TT = TQ // 128       # query t-tiles
    ICS = 512            # i-chunk size (one attention pass per half)
    ICN = TQ // ICS
    assert DH == 64 and F % 128 == 0 and D % 128 == 0
    assert TQ % 128 == 0 and TK % 128 == 0 and ICN == 2 and FC == 8
    SCALE_QK = 1.0 / DH ** 0.5

    _pin_act_tables()
    nc = bacc.Bacc("TRN2", target_bir_lowering=False, debug=debug,
                   num_devices=n_cores)

    # ---- DRAM I/O (projection inputs pre-swizzled on the host to
    # partition-major [128, chunk, cols], split head/rest so the whole
    # attend(0,0) dependency set is a handful of big DMAs) ----
    hTqh_d = nc.dram_tensor("hTqh", [128, DC, ICS], FP8, kind="ExternalInput")
    hTqr_d = nc.dram_tensor("hTqr", [128, DC, TQ - ICS], FP8,
                            kind="ExternalInput")
    hTk_d = nc.dram_tensor("hTk", [128, DC, TK], FP8, kind="ExternalInput")
    hq_d = nc.dram_tensor("hq", [TQ, D], F32, kind="ExternalInput")
    wqh_d = nc.dram_tensor("wqh", [128, DC, 128], FP8, kind="ExternalInput")
    wqr_d = nc.dram_tensor("wqr", [128, DC, F - 128], FP8,
                           kind="ExternalInput")
    wkh_d = nc.dram_tensor("wkh", [128, DC, 128], FP8, kind="ExternalInput")
    wkr_d = nc.dram_tensor("wkr", [128, DC, F - 128], FP8,
                           kind="ExternalInput")
    wvh_d = nc.dram_tensor("wvh", [128, DC, 512], FP8, kind="ExternalInput")
    wvr_d = nc.dram_tensor("wvr", [128, DC, F - 512], FP8,
                           kind="ExternalInput")
    woT_d = nc.dram_tensor("woT", [128, FC, D], FP8, kind="ExternalInput")
    mb_d = nc.dram_tensor("maskbias", [128, KC], F32, kind="ExternalInput")
    g_d = nc.dram_tensor("g_rep", [128, D], F32, kind="ExternalInput")
    b_d = nc.dram_tensor("b_rep", [128, D], F32, kind="ExternalInput")
    out_d = nc.dram_tensor("out", [TQ, D], F32, kind="ExternalOutput")

    with tile.TileContext(nc) as tc:
        with (
            tc.tile_pool(name="hpool", bufs=1) as hpool,
            tc.tile_pool(name="wts", bufs=1) as wts,
            tc.tile_pool(name="acts", bufs=1) as acts,
            tc.tile_pool(name="small", bufs=1) as small,
            tc.tile_pool(name="exps", bufs=4) as expp,
            tc.tile_pool(name="evac", bufs=2) as evac,
            tc.tile_pool(name="epi", bufs=4) as epi,
            tc.tile_pool(name="psS", bufs=2, space="PSUM") as psS,
            tc.tile_pool(name="psPV", bufs=1, space="PSUM") as psPV,
            tc.tile_pool(name="psO", bufs=2, space="PSUM") as psO,
        ):
            # ---- persistent SBUF tiles (projection operands in fp8;
            # pair axis for DoubleRow is the chunk dim of the 3D tiles) ----
            hTq = hpool.tile([128, DC, TQ], FP8, tag="htq")
            hTk = hpool.tile([128, DC, TK], FP8, tag="htk")
            # wq and wo share a ring slot: wo's DMA is emitted after the
            # last Q-proj chain so it waits for wq's death naturally.
            wqT = wts.tile([128, DC, F], FP8, tag="wqo")
            wkT = wts.tile([128, DC, F], FP8, tag="wk")
            wvT = wts.tile([128, DC, F], FP8, tag="wv")
            QT = acts.tile([128, FC * TQ], BF16, tag="qt")
            KT = acts.tile([128, FC * TK], BF16, tag="kt")
            V = acts.tile([128, KC * F], BF16, tag="v")
            AVT = acts.tile([128, FC, TQ], FP8, tag="avt")
            ones = small.tile([128, 64], BF16, tag="ones")
            mb = small.tile([128, KC], F32, tag="mb")
            eps_t = small.tile([128, 1], F32, tag="eps")
            g_re = small.tile([128, D], F32, tag="g")
            b_re = small.tile([128, D], F32, tag="b")

            nc.vector.memset(ones[:], 1.0)
            nc.vector.memset(eps_t[:], 1e-5)
            nc.sync.dma_start(mb[:], mb_d[:])
            # Prologue: one big DMA per piece, spread over two DGE
            # queues (each dma_start costs ~650ns of issuing-engine time,
            # so instruction count -- not just bytes -- sets the head).
            nc.sync.dma_start(hTk[:], hTk_d[:])
            nc.gpsimd.dma_start(hTq[:, :, 0:ICS], hTqh_d[:])
            nc.sync.dma_start(wkT[:, :, 0:128], wkh_d[:])
            nc.gpsimd.dma_start(wvT[:, :, 0:512], wvh_d[:])
            nc.sync.dma_start(wqT[:, :, 0:128], wqh_d[:])
            # remainders (needed by fillers from ~iter0-mid onward)
            nc.gpsimd.dma_start(hTq[:, :, ICS:TQ], hTqr_d[:])
            nc.sync.dma_start(wqT[:, :, 128:F], wqr_d[:])
            nc.gpsimd.dma_start(wkT[:, :, 128:F], wkr_d[:])
            nc.sync.dma_start(wvT[:, :, 512:F], wvr_d[:])

            # ---- projection chain emitters (each is one filler unit).
            # All four projections run fp8 DoubleRow: each matmul contracts
            # a PAIR of 128-row chunks (lhsT/rhs get a [128, 2, *] view). ----
            DP = DC // 2

            def q_chain(hp, ic):
                io = ic * ICS
                ps = psO.tile([128, ICS], F32, tag="po")
                for p in range(DP):
                    nc.tensor.matmul(
                        ps[:],
                        wqT[:, 2 * p:2 * p + 2, hp * 128:(hp + 1) * 128],
                        hTq[:, 2 * p:2 * p + 2, io:io + ICS],
                        start=(p == 0), stop=(p == DP - 1), perf_mode=DR)
                nc.vector.tensor_copy(
                    QT[:, hp * TQ + io: hp * TQ + io + ICS], ps[:])

            def k_chain(hp, t0, tn):
                ps = psO.tile([128, tn], F32, tag="po")
                for p in range(DP):
                    nc.tensor.matmul(
                        ps[:],
                        wkT[:, 2 * p:2 * p + 2, hp * 128:(hp + 1) * 128],
                        hTk[:, 2 * p:2 * p + 2, t0:t0 + tn],
                        start=(p == 0), stop=(p == DP - 1), perf_mode=DR)
                nc.vector.tensor_copy(
                    KT[:, hp * TK + t0: hp * TK + t0 + tn], ps[:])

            def v_chain(jc, f0, fn):
                ps = psO.tile([128, fn], F32, tag="po")
                for p in range(DP):
                    nc.tensor.matmul(
                        ps[:],
                        hTk[:, 2 * p:2 * p + 2, jc * 128:(jc + 1) * 128],
                        wvT[:, 2 * p:2 * p + 2, f0:f0 + fn],
                        start=(p == 0), stop=(p == DP - 1), perf_mode=DR)
                nc.vector.tensor_copy(
                    V[:, jc * F + f0: jc * F + f0 + fn], ps[:])

            # ---- LayerNorm epilogue, split into stages so the tail can
            # be emitted stage-major (per-engine batches pipeline across
            # tiles instead of HOL-blocking in the strict FIFOs). ----
            def oln_stage_oproj(tt):
                hqt = epi.tile([128, D], F32, tag="hqt")
                nc.sync.dma_start(hqt[:], hq_d[tt * 128:(tt + 1) * 128, :])
                x = epi.tile([128, D], F32, tag="x")
                st = epi.tile([128, 8], F32, tag="stats")
                for ci, (d0, dn) in enumerate(_chunks(D, 512)):
                    ps = psO.tile([128, dn], F32, tag="po")
                    for p in range(FC // 2):
                        nc.tensor.matmul(
                            ps[:],
                            AVT[:, 2 * p:2 * p + 2, tt * 128:(tt + 1) * 128],
                            woT[:, 2 * p:2 * p + 2, d0:d0 + dn],
                            start=(p == 0), stop=(p == FC // 2 - 1),
                            perf_mode=DR)
                    nc.vector.scalar_tensor_tensor(
                        x[:, d0:d0 + dn], ps[:], 1.0, hqt[:, d0:d0 + dn],
                        op0=ALU.mult, op1=ALU.add,
                        accum_out=(st[:, 0:1] if ci == 0 else st[:, 1:2]))
                return x, st

            def oln_stage_sq(x, st, dve=False):
                xc = epi.tile([128, D], F32, tag="xc")
                if dve:
                    nc.vector.scalar_tensor_tensor(
                        xc[:], x[:], 1.0, x[:], op0=ALU.mult, op1=ALU.mult,
                        accum_out=st[:, 3:4])
                else:
                    nc.scalar.activation(xc[:], x[:], AF.Square,
                                         accum_out=st[:, 3:4])
                return xc

            def oln_stage_stats1(st):
                s1a, s1b, mu, mu2 = (st[:, 0:1], st[:, 1:2],
                                     st[:, 2:3], st[:, 1:2])
                nc.vector.tensor_tensor(s1a, s1a, s1b, op=ALU.add)
                nc.vector.tensor_scalar(mu, s1a, 1.0 / D, None, op0=ALU.mult)
                nc.vector.tensor_tensor(mu2, mu, mu, op=ALU.mult)
                nc.vector.scalar_tensor_tensor(
                    st[:, 4:5], st[:, 3:4], 1.0 / D, mu2,
                    op0=ALU.mult, op1=ALU.subtract)
                # -mu into slot 3 (s2 is dead once var is computed); used
                # by the DVE normalize variant.
                nc.vector.tensor_scalar(st[:, 3:4], mu, -1.0, None,
                                        op0=ALU.mult)

            def oln_stage_rstd(st):
                nc.scalar.activation(st[:, 6:7], st[:, 4:5], AF.Ln,
                                     bias=eps_t[:])
                nc.scalar.activation(st[:, 5:6], st[:, 6:7], AF.Exp,
                                     scale=-0.5)

            def oln_stage_nmr(st):
                nc.vector.scalar_tensor_tensor(
                    st[:, 7:8], st[:, 2:3], -1.0, st[:, 5:6],
                    op0=ALU.mult, op1=ALU.mult)

            def oln_stage_norm(x, xc, st, dve=False):
                if dve:
                    # (x - mu) * rstd as one 2-scalar DVE op (2x_2P mode)
                    nc.vector.tensor_scalar(
                        xc[:], x[:], st[:, 3:4], st[:, 5:6],
                        op0=ALU.add, op1=ALU.mult)
                else:
                    nc.scalar.activation(xc[:], x[:], AF.Identity,
                                         bias=st[:, 7:8], scale=st[:, 5:6])

            def oln_stage_gb(tt, xc):
                if not triv_gb:
                    nc.vector.tensor_tensor(xc[:], xc[:], g_re[:],
                                            op=ALU.mult)
                    nc.vector.tensor_tensor(xc[:], xc[:], b_re[:],
                                            op=ALU.add)
                nc.sync.dma_start(out_d[tt * 128:(tt + 1) * 128, :], xc[:])

            def oln_tile(tt):
                # used under the last attend: keep ACT free for exp by
                # running the heavy Square/normalize passes on DVE
                x, st = oln_stage_oproj(tt)
                xc = oln_stage_sq(x, st, dve=True)
                oln_stage_stats1(st)
                oln_stage_rstd(st)
                oln_stage_norm(x, xc, st, dve=True)
                oln_stage_gb(tt, xc)

            def oln_tail(tts):
                xs, xcs, sts = {}, {}, {}
                dv = {tt: (k % 2 == 1) for k, tt in enumerate(tts)}
                for tt in tts:
                    xs[tt], sts[tt] = oln_stage_oproj(tt)
                for tt in tts:
                    xcs[tt] = oln_stage_sq(xs[tt], sts[tt], dve=dv[tt])
                for tt in tts:
                    oln_stage_stats1(sts[tt])
                for tt in tts:
                    oln_stage_rstd(sts[tt])
                for tt in tts:
                    if not dv[tt]:
                        oln_stage_nmr(sts[tt])
                for tt in tts:
                    oln_stage_norm(xs[tt], xcs[tt], sts[tt], dve=dv[tt])
                for tt in tts:
                    oln_stage_gb(tt, xcs[tt])

            # ---- attention for one (head-pair, i-chunk), with fillers.
            # Software-pipelined: S-pairs run TWO j-steps ahead of their
            # exp, and PV/den consume e one exp behind, so neither PE nor
            # ACT ever waits on the other in steady state (psS bufs=2 and
            # the e-ring provide exactly the needed decoupling). ----
            def attend(hp, ic, fillers):
                io = ic * ICS
                h0, h1 = 2 * hp, 2 * hp + 1
                pv = psPV.tile([128, 2 * ICS], F32, tag="pv")

                def s_pair(jc):
                    s = psS.tile([128, 2 * ICS], F32, tag="s")
                    nc.tensor.matmul(
                        s[:, 0:ICS],
                        KT[0:64, hp * TK + jc * 128: hp * TK + (jc + 1) * 128],
                        QT[0:64, hp * TQ + io: hp * TQ + io + ICS],
                        start=True, stop=True, tile_position=(0, 0))
                    nc.tensor.matmul(
                        s[:, ICS:2 * ICS],
                        KT[64:128, hp * TK + jc * 128: hp * TK + (jc + 1) * 128],
                        QT[64:128, hp * TQ + io: hp * TQ + io + ICS],
                        start=True, stop=True, tile_position=(64, 0))
                    return s

                def exp_op(s):
                    e = expp.tile([128, 2 * ICS], BF16, tag="e")
                    # wq is sent UNSCALED (fp8 can't hold wq/8 — subnormal
                    # territory); 1/sqrt(DH) rides ACT's free affine scale.
                    nc.scalar.activation(e[:], s[:], AF.Exp, scale=SCALE_QK,
                                         bias=mb[:, jc_of[id(s)]:jc_of[id(s)] + 1])
                    return e

                def pv_den(jc, e):
                    st_, sp = (jc == 0), (jc == KC - 1)

                    def pv_pair():
                        nc.tensor.matmul(
                            pv[0:64, 0:ICS],
                            V[:, jc * F + h0 * DH: jc * F + (h0 + 1) * DH],
                            e[:, 0:ICS], start=st_, stop=sp,
                            tile_position=(0, 0), skip_group_check=True)
                        nc.tensor.matmul(
                            pv[64:128, 0:ICS],
                            V[:, jc * F + h1 * DH: jc * F + (h1 + 1) * DH],
                            e[:, ICS:2 * ICS], start=st_, stop=sp,
                            tile_position=(0, 64), skip_group_check=True)

                    def den_pair():
                        nc.tensor.matmul(
                            pv[0:64, ICS:2 * ICS], ones[:, 0:64],
                            e[:, 0:ICS], start=st_, stop=sp,
                            tile_position=(0, 0), skip_group_check=True)
                        nc.tensor.matmul(
                            pv[64:128, ICS:2 * ICS], ones[:, 0:64],
                            e[:, ICS:2 * ICS], start=st_, stop=sp,
                            tile_position=(0, 64), skip_group_check=True)

                    # at the attend boundary (jc 0) the den bank is freed
                    # by the previous attend's reciprocal ~0.7us before
                    # the multiply frees the PV bank -- start with den.
                    if st_:
                        den_pair(); pv_pair()
                    else:
                        pv_pair(); den_pair()

                jc_of = {}
                s0 = s_pair(0); jc_of[id(s0)] = 0
                e_prev = exp_op(s0)
                s_next = s_pair(1); jc_of[id(s_next)] = 1
                for jc in range(KC):
                    if jc + 1 < KC:
                        e_next = exp_op(s_next)
                    if jc + 2 < KC:
                        s_next = s_pair(jc + 2); jc_of[id(s_next)] = jc + 2
                    if fillers:
                        fillers.popleft()()
                    pv_den(jc, e_prev)
                    if jc + 1 < KC:
                        e_prev = e_next
                if fillers:
                    fillers.popleft()()
                # normalize straight out of PSUM: recip(den) then
                # AVT = pvP * rec -- the two reads free psPV for the
                # next attend; no evacuation copy.
                rec = evac.tile([128, ICS], F32, tag="rec")
                nc.vector.reciprocal_approx_fast(rec[:], pv[:, ICS:2 * ICS])
                nc.vector.tensor_tensor(
                    AVT[:, hp, io:io + ICS],
                    pv[:, 0:ICS], rec[:], op=ALU.mult)

            # ---- build per-iteration filler lists ----
            from collections import deque
            k_chunks = _chunks(TK, 512)
            # V chains: group A covers head-pairs 0-3 (f 0:512), B covers
            # 4-7. A(jc0) + Q0/K0 run in the prologue; A(jc>=1) fill iter 0
            # just-in-time (filler jc-1 lands between exp(jc-1) and PV(jc-1),
            # i.e. before PV(jc) needs V[jc]).  B spreads over iters 1-3.
            fillers = {hp: deque() for hp in range(8)}
            for jc in range(1, KC):
                fillers[0].append(lambda jc=jc: v_chain(jc, 0, 512))
            vb = deque(range(KC))
            for hp in range(1, 4):
                for _ in range(3):
                    if vb:
                        jc = vb.popleft()
                        fillers[hp].append(
                            lambda jc=jc: v_chain(jc, 512, 512))
            for hp in range(7):
                fillers[hp].append(lambda hp=hp: q_chain(hp + 1, 0))
                fillers[hp].append(lambda hp=hp: q_chain(hp + 1, 1))
                for t0, tn in k_chunks:
                    fillers[hp].append(
                        lambda hp=hp, t0=t0, tn=tn: k_chain(hp + 1, t0, tn))

            # ---- prologue: minimal deps for attend(0, ic0), emitted in
            # DMA-arrival order so the PE FIFO is never head-blocked ----
            for t0, tn in k_chunks:
                k_chain(0, t0, tn)
            v_chain(0, 0, 512)
            q_chain(0, 0)
            fillers[0].appendleft(lambda: q_chain(0, 1))

            # ---- main pipeline ----
            for hp in range(8):
                attend(hp, 0, fillers[hp])
                if hp == 6:
                    # wo reuses wq's ring slot; its DMA is emitted after the
                    # last wq consumer (Q[7] chains, queued in fillers[6])
                    # so the ring dependency resolves without deadlock.
                    woT = wts.tile([128, FC, D], FP8, tag="wqo")
                    nc.sync.dma_start(woT[:], woT_d[:])
                    if not triv_gb:
                        nc.sync.dma_start(g_re[:], g_d[:])
                        nc.sync.dma_start(b_re[:], b_d[:])
                if hp == 7:
                    ic1_fillers = deque(
                        [lambda tt=tt: oln_tile(tt) for tt in range(4)])
                    attend(hp, 1, ic1_fillers)
                    while ic1_fillers:
                        ic1_fillers.popleft()()
                else:
                    attend(hp, 1, fillers[hp])
                while fillers[hp]:
                    fillers[hp].popleft()()

            # ---- tail: second query-half epilogues, stage-major ----
            oln_tail(list(range(4, TT)))

    nc.compile()
    return nc


def choose_tk(attn_mask):
    """Compacted key count: max unmasked count over batches, ceil to 128."""
    m = np.asarray(attn_mask)
    counts = (~m).sum(axis=0)
    tk = int(((int(counts.max()) + 127) // 128) * 128)
    return max(tk, 128)


def host_prep_core(c, tk, h, attn_mask, wq, wkv, wo, ln_g, ln_b, NH=16, DH=64):
    """Build the per-core input map (numpy) for core c."""
    T, B, D = h.shape
    F = NH * DH
    TQ = T // 2
    KC = tk // 128
    b, qh = c // 2, c % 2
    bf = ml_dtypes.bfloat16
    f8 = ml_dtypes.float8_e4m3fn
    hb = np.roll(np.asarray(h[:, b, :], dtype=np.float32), -qh * TQ, axis=0)
    maskb = np.roll(np.asarray(attn_mask[:, b]), -qh * TQ)
    idx = np.nonzero(~maskb)[0]
    nk = idx.shape[0]
    assert nk <= tk
    idxp = np.concatenate([idx, np.zeros(tk - nk, np.int64)])
    DC = D // 128
    ICS = 512

    def swiz(a):  # [D, X] -> [128, DC, X] partition-major
        return np.ascontiguousarray(
            a.reshape(DC, 128, a.shape[1]).transpose(1, 0, 2))

    hbT = hb.T.astype(f8)                                   # [D, T]
    m = {}
    hTq = swiz(hbT[:, :TQ])
    m["hTqh"], m["hTqr"] = (np.ascontiguousarray(hTq[:, :, :ICS]),
                            np.ascontiguousarray(hTq[:, :, ICS:]))
    m["hTk"] = swiz(hbT[:, idxp])
    m["hq"] = np.ascontiguousarray(hb[:TQ])                 # [TQ, D] f32
    # wq goes UNSCALED (wq/8 would be fp8-subnormal); the kernel applies
    # 1/sqrt(DH) inside the Exp activation instead.
    wq8 = swiz(wq.T.astype(f8))
    m["wqh"], m["wqr"] = (np.ascontiguousarray(wq8[:, :, :128]),
                          np.ascontiguousarray(wq8[:, :, 128:]))
    wk8 = swiz(wkv[:F].T.astype(f8))
    m["wkh"], m["wkr"] = (np.ascontiguousarray(wk8[:, :, :128]),
                          np.ascontiguousarray(wk8[:, :, 128:]))
    wv8 = swiz(wkv[F:].T.astype(f8))
    m["wvh"], m["wvr"] = (np.ascontiguousarray(wv8[:, :, :512]),
                          np.ascontiguousarray(wv8[:, :, 512:]))
    m["woT"] = swiz(wo.T.astype(f8))
    mbias = np.full(tk, NEG_BIG, np.float32)
    mbias[:nk] = 0.0
    m["maskbias"] = np.ascontiguousarray(mbias.reshape(KC, 128).T)
    m["g_rep"] = np.ascontiguousarray(
        np.broadcast_to(np.asarray(ln_g, np.float32), (128, D)))
    m["b_rep"] = np.ascontiguousarray(
        np.broadcast_to(np.asarray(ln_b, np.float32), (128, D)))
    return m

# ======================================================================
# Host-side runner: shard, compile (cached), execute on 8 cores, gather.
# ======================================================================
_NC_CACHE = {}
LAST_RESULT = None  # BassKernelResults of the most recent kernel() call


def _get_nc(T, TQ, TK, D, NH, DH, triv_gb):
    key = (T, TQ, TK, D, NH, DH, triv_gb)
    if key not in _NC_CACHE:
        _NC_CACHE[key] = build_nc(T, TQ, TK, D, NH, DH, n_cores=8,
                                  debug=False, triv_gb=triv_gb)
    return _NC_CACHE[key]


def kernel(h, attn_mask, wq, wkv, wo, ln_g, ln_b):
    """Full-input MultiHeadAttn forward on 8 NeuronCores.

    h: [T, B, D] f32; attn_mask: [T, B] bool (True = masked key);
    wq: [F, D]; wkv: [2F, D]; wo: [D, F]; ln_g/ln_b: [D].
    Returns [T, B, D] f32 = layer_norm(h + attn(h)).
    """
    from concourse.bass_utils import run_bass_kernel_spmd
    global LAST_RESULT

    h = np.asarray(h)
    attn_mask = np.asarray(attn_mask)
    wq = np.asarray(wq, np.float32)
    wkv = np.asarray(wkv, np.float32)
    wo = np.asarray(wo, np.float32)
    ln_g = np.asarray(ln_g, np.float32)
    ln_b = np.asarray(ln_b, np.float32)

    T, B, D = h.shape
    NH = 16
    DH = wq.shape[0] // NH
    assert 2 * B == 8, "sharding assumes batch 4 over 8 cores"
    TQ = T // 2
    TK = min(choose_tk(attn_mask), T)

    triv_gb = bool(np.all(ln_g == 1.0) and np.all(ln_b == 0.0))
    nc = _get_nc(T, TQ, TK, D, NH, DH, triv_gb)
    in_maps = [host_prep_core(c, TK, h, attn_mask, wq, wkv, wo, ln_g, ln_b,
                              NH=NH, DH=DH) for c in range(8)]
    res = run_bass_kernel_spmd(nc, in_maps, core_ids=list(range(8)))
    LAST_RESULT = res

    out = np.empty((T, B, D), np.float32)
    for c in range(8):
        b, qh = c // 2, c % 2
        out[qh * TQ:(qh + 1) * TQ, b, :] = res.results[c]["out"]
    return out


# revision 32
# speedup vs baseline: 1.0129x; 1.0129x over previous
"""Multi-head attention Bass/Tile kernel for TRN2, 8-core SPMD.

Sharding: core c handles batch b = c//2, query-half qh = c%2. The host
rotates the token axis per core so query rows sit at [0:TQ] (attention is
key-permutation invariant), and gathers the unmasked keys (mask compaction)
so K/V projection + attention only touch TK <= T key tokens.

Final architecture (378us baseline -> ~229us):
  - hp-major single pass: for each head-pair, attention over both query
    halves, with Q/K/V projection chains for later head-pairs popped as
    "filler" PE work inside the attention loop.
  - attention inner loop is SOFTWARE PIPELINED: S-pairs run two j-steps
    ahead of their exp, PV/den consume e one exp behind, so PE and ACT
    never block each other in steady state (exp cadence = back-to-back
    ~1.0us; PE paces at ~1.2us/j-step).
  - all four projections (Q/K/V/O) run fp8 e4m3 with DoubleRow: each
    matmul contracts a PAIR of 128-chunks, halving projection PE time.
    wq is sent UNSCALED (wq/8 is fp8-subnormal); 1/sqrt(DH) rides the
    Exp activation's free affine scale. rel err ~1e-3 (vs 6.6e-5 bf16).
  - reciprocal_approx_fast for the softmax denominator, reading pvP/den
    straight from PSUM (no evacuation copy).
  - LayerNorm: mean rides the residual-add stt accum_out, var via one
    ACT Square+accum, normalize via one ACT Identity with per-partition
    scale/bias; the g/b multiply-add is skipped entirely when the host
    detects ln_g==1, ln_b==0 (general path kept otherwise).
  - epilogues: first-half tiles run under the last head-pair's second
    attention pass; the 4 tail tiles are emitted STAGE-MAJOR so their
    chains pipeline across the strict per-engine FIFOs.
  - DMA: inputs host-swizzled to partition-major [128, chunk, cols] and
    split head/rest, one dma_start each, spread over the Sync and GpSimd
    DGE queues (each dma_start costs ~650ns of issuing-engine time).
PSUM: 2x score tiles (4 banks) + pv|den tile (2) + proj/o-proj ring (2).

Matmul layouts (out = lhsT.T @ rhs, contraction on partitions):
  QT/KT [F, *] bf16 : lhsT=w*T [D,F] chunks, rhs=hT* [D,*] chunks
  V     [TK, F] bf16: lhsT=hTk chunk [D, t128], rhs=wvT [D, F]
  S^T   [j, (h0 i512 | h1 i512)] psum 2 banks, row-split tile_position
  exp   one ACT op per j-tile: [128, 1024] psum->sbuf bf16, bias=maskbias
  PV+den one psum tile [128, (pv 512 | den 512)]: V cols / ones lhsT
  O     [t, D] psum : lhsT=AVT [f, t128], rhs=woT [f, D] in 512 chunks
"""
import numpy as np
import ml_dtypes

import concourse.bass as bass
import concourse.tile as tile
from concourse import bacc, mybir

F32 = mybir.dt.float32
BF16 = mybir.dt.bfloat16
FP8 = mybir.dt.float8e4
DR = mybir.MatmulPerfMode.DoubleRow
AF = mybir.ActivationFunctionType
ALU = mybir.AluOpType

NEG_BIG = -1.0e30


def _pin_act_tables():
    """Force every ACT func we use (Exp, Ln, Square, Identity, Copy) to
    resolve to the single `natural_log_exp_and_others` table set, so the
    kernel does exactly one ACT_TABLE_LOAD instead of thrashing (~2.6us
    per switch). Preserves dict order (set ids are positional)."""
    import concourse.hw_specs as hw_specs
    if getattr(hw_specs, "_mha_tables_pinned", False):
        return
    orig = hw_specs.get_activation_tables

    def patched(module_arch):
        tabs = orig(module_arch)
        pin = "natural_log_exp_and_others"
        if pin in tabs:
            pinned_funcs = tabs[pin]
            for name, fns in tabs.items():
                if name != pin:
                    tabs[name] = fns - pinned_funcs
        return tabs

    hw_specs.get_activation_tables = patched
    import concourse.bacc as bacc_mod
    bacc_mod.get_activation_tables = patched
    hw_specs._mha_tables_pinned = True


def _chunks(total, step):
    out = []
    off = 0
    while off < total:
        out.append((off, min(step, total - off)))
        off += step
    return out


def build_nc(T, TQ, TK, D, NH, DH, n_cores=8, debug=False, triv_gb=False):
    """Build the single-core SPMD Bass program. TK = compacted key count."""
    F = NH * DH
    DC = D // 128        # D contraction chunks
    FC = F // 128        # feature chunks (2 heads per chunk, DH=64)
    KC = TK // 128       # key tiles
    TT = TQ // 128       # query t-tiles
    ICS = 512            # i-chunk size (one attention pass per half)
    ICN = TQ // ICS
    assert DH == 64 and F % 128 == 0 and D % 128 == 0
    assert TQ % 128 == 0 and TK % 128 == 0 and ICN == 2 and FC == 8
    SCALE_QK = 1.0 / DH ** 0.5

    _pin_act_tables()
    nc = bacc.Bacc("TRN2", target_bir_lowering=False, debug=debug,
                   num_devices=n_cores)

    # ---- DRAM I/O (projection inputs pre-swizzled on the host to
    # partition-major [128, chunk, cols], split head/rest so the whole
    # attend(0,0) dependency set is a handful of big DMAs) ----
    hTqh_d = nc.dram_tensor("hTqh", [128, DC, ICS], FP8, kind="ExternalInput")
    hTqr_d = nc.dram_tensor("hTqr", [128, DC, TQ - ICS], FP8,
                            kind="ExternalInput")
    hTk_d = nc.dram_tensor("hTk", [128, DC, TK], FP8, kind="ExternalInput")
    hq_d = nc.dram_tensor("hq", [TQ, D], F32, kind="ExternalInput")
    wqh_d = nc.dram_tensor("wqh", [128, DC, 128], FP8, kind="ExternalInput")
    wqr_d = nc.dram_tensor("wqr", [128, DC, F - 128], FP8,
                           kind="ExternalInput")
    wkh_d = nc.dram_tensor("wkh", [128, DC, 128], FP8, kind="ExternalInput")
    wkr_d = nc.dram_tensor("wkr", [128, DC, F - 128], FP8,
                           kind="ExternalInput")
    wvh_d = nc.dram_tensor("wvh", [128, DC, 512], FP8, kind="ExternalInput")
    wvr_d = nc.dram_tensor("wvr", [128, DC, F - 512], FP8,
                           kind="ExternalInput")
    woT_d = nc.dram_tensor("woT", [128, FC, D], FP8, kind="ExternalInput")
    mb_d = nc.dram_tensor("maskbias", [128, KC], F32, kind="ExternalInput")
    g_d = nc.dram_tensor("g_rep", [128, D], F32, kind="ExternalInput")
    b_d = nc.dram_tensor("b_rep", [128, D], F32, kind="ExternalInput")
    out_d = nc.dram_tensor("out", [TQ, D], F32, kind="ExternalOutput")

    with tile.TileContext(nc) as tc:
        with (
            tc.tile_pool(name="hpool", bufs=1) as hpool,
            tc.tile_pool(name="wts", bufs=1) as wts,
            tc.tile_pool(name="acts", bufs=1) as acts,
            tc.tile_pool(name="small", bufs=1) as small,
            tc.tile_pool(name="exps", bufs=4) as expp,
            tc.tile_pool(name="evac", bufs=2) as evac,
            tc.tile_pool(name="epi", bufs=4) as epi,
            tc.tile_pool(name="psS", bufs=2, space="PSUM") as psS,
            tc.tile_pool(name="psPV", bufs=1, space="PSUM") as psPV,
            tc.tile_pool(name="psO", bufs=2, space="PSUM") as psO,
        ):
            # ---- persistent SBUF tiles (projection operands in fp8;
            # pair axis for DoubleRow is the chunk dim of the 3D tiles) ----
            hTq = hpool.tile([128, DC, TQ], FP8, tag="htq")
            hTk = hpool.tile([128, DC, TK], FP8, tag="htk")
            # wq and wo share a ring slot: wo's DMA is emitted after the
            # last Q-proj chain so it waits for wq's death naturally.
            wqT = wts.tile([128, DC, F], FP8, tag="wqo")
            wkT = wts.tile([128, DC, F], FP8, tag="wk")
            wvT = wts.tile([128, DC, F], FP8, tag="wv")
            QT = acts.tile([128, FC * TQ], BF16, tag="qt")
            KT = acts.tile([128, FC * TK], BF16, tag="kt")
            V = acts.tile([128, KC * F], BF16, tag="v")
            AVT = acts.tile([128, FC, TQ], FP8, tag="avt")
            ones = small.tile([128, 64], BF16, tag="ones")
            mb = small.tile([128, KC], F32, tag="mb")
            eps_t = small.tile([128, 1], F32, tag="eps")
            g_re = small.tile([128, D], F32, tag="g")
            b_re = small.tile([128, D], F32, tag="b")

            nc.vector.memset(ones[:], 1.0)
            nc.vector.memset(eps_t[:], 1e-5)
            nc.sync.dma_start(mb[:], mb_d[:])
            # Prologue: one big DMA per piece, spread over two DGE
            # queues (each dma_start costs ~650ns of issuing-engine time,
            # so instruction count -- not just bytes -- sets the head).
            nc.sync.dma_start(hTk[:], hTk_d[:])
            nc.gpsimd.dma_start(hTq[:, :, 0:ICS], hTqh_d[:])
            nc.sync.dma_start(wkT[:, :, 0:128], wkh_d[:])
            nc.gpsimd.dma_start(wvT[:, :, 0:512], wvh_d[:])
            nc.sync.dma_start(wqT[:, :, 0:128], wqh_d[:])
            # remainders (needed by fillers from ~iter0-mid onward)
            nc.gpsimd.dma_start(hTq[:, :, ICS:TQ], hTqr_d[:])
            nc.sync.dma_start(wqT[:, :, 128:F], wqr_d[:])
            nc.gpsimd.dma_start(wkT[:, :, 128:F], wkr_d[:])
            nc.sync.dma_start(wvT[:, :, 512:F], wvr_d[:])

            # ---- projection chain emitters (each is one filler unit).
            # All four projections run fp8 DoubleRow: each matmul contracts
            # a PAIR of 128-row chunks (lhsT/rhs get a [128, 2, *] view). ----
            DP = DC // 2

            def q_chain(hp, ic):
                io = ic * ICS
                ps = psO.tile([128, ICS], F32, tag="po")
                for p in range(DP):
                    nc.tensor.matmul(
                        ps[:],
                        wqT[:, 2 * p:2 * p + 2, hp * 128:(hp + 1) * 128],
                        hTq[:, 2 * p:2 * p + 2, io:io + ICS],
                        start=(p == 0), stop=(p == DP - 1), perf_mode=DR)
                nc.vector.tensor_copy(
                    QT[:, hp * TQ + io: hp * TQ + io + ICS], ps[:])

            def k_chain(hp, t0, tn):
                ps = psO.tile([128, tn], F32, tag="po")
                for p in range(DP):
                    nc.tensor.matmul(
                        ps[:],
                        wkT[:, 2 * p:2 * p + 2, hp * 128:(hp + 1) * 128],
                        hTk[:, 2 * p:2 * p + 2, t0:t0 + tn],
                        start=(p == 0), stop=(p == DP - 1), perf_mode=DR)
                nc.vector.tensor_copy(
                    KT[:, hp * TK + t0: hp * TK + t0 + tn], ps[:])

            def v_chain(jc, f0, fn):
                ps = psO.tile([128, fn], F32, tag="po")
                for p in range(DP):
                    nc.tensor.matmul(
                        ps[:],
                        hTk[:, 2 * p:2 * p + 2, jc * 128:(jc + 1) * 128],
                        wvT[:, 2 * p:2 * p + 2, f0:f0 + fn],
                        start=(p == 0), stop=(p == DP - 1), perf_mode=DR)
                nc.vector.tensor_copy(
                    V[:, jc * F + f0: jc * F + f0 + fn], ps[:])

            # ---- LayerNorm epilogue, split into stages so the tail can
            # be emitted stage-major (per-engine batches pipeline across
            # tiles instead of HOL-blocking in the strict FIFOs). ----
            def oln_stage_oproj(tt):
                hqt = epi.tile([128, D], F32, tag="hqt")
                nc.sync.dma_start(hqt[:], hq_d[tt * 128:(tt + 1) * 128, :])
                x = epi.tile([128, D], F32, tag="x")
                st = epi.tile([128, 8], F32, tag="stats")
                for ci, (d0, dn) in enumerate(_chunks(D, 512)):
                    ps = psO.tile([128, dn], F32, tag="po")
                    for p in range(FC // 2):
                        nc.tensor.matmul(
                            ps[:],
                            AVT[:, 2 * p:2 * p + 2, tt * 128:(tt + 1) * 128],
                            woT[:, 2 * p:2 * p + 2, d0:d0 + dn],
                            start=(p == 0), stop=(p == FC // 2 - 1),
                            perf_mode=DR)
                    nc.vector.scalar_tensor_tensor(
                        x[:, d0:d0 + dn], ps[:], 1.0, hqt[:, d0:d0 + dn],
                        op0=ALU.mult, op1=ALU.add,
                        accum_out=(st[:, 0:1] if ci == 0 else st[:, 1:2]))
                return x, st

            def oln_stage_sq(x, st, dve=False):
                xc = epi.tile([128, D], F32, tag="xc")
                if dve:
                    nc.vector.scalar_tensor_tensor(
                        xc[:], x[:], 1.0, x[:], op0=ALU.mult, op1=ALU.mult,
                        accum_out=st[:, 3:4])
                else:
                    nc.scalar.activation(xc[:], x[:], AF.Square,
                                         accum_out=st[:, 3:4])
                return xc

            def oln_stage_stats1(st):
                s1a, s1b, mu, mu2 = (st[:, 0:1], st[:, 1:2],
                                     st[:, 2:3], st[:, 1:2])
                nc.vector.tensor_tensor(s1a, s1a, s1b, op=ALU.add)
                nc.vector.tensor_scalar(mu, s1a, 1.0 / D, None, op0=ALU.mult)
                nc.vector.tensor_tensor(mu2, mu, mu, op=ALU.mult)
                nc.vector.scalar_tensor_tensor(
                    st[:, 4:5], st[:, 3:4], 1.0 / D, mu2,
                    op0=ALU.mult, op1=ALU.subtract)
                # -mu into slot 3 (s2 is dead once var is computed); used
                # by the DVE normalize variant.
                nc.vector.tensor_scalar(st[:, 3:4], mu, -1.0, None,
                                        op0=ALU.mult)

            def oln_stage_rstd(st):
                nc.scalar.activation(st[:, 6:7], st[:, 4:5], AF.Ln,
                                     bias=eps_t[:])
                nc.scalar.activation(st[:, 5:6], st[:, 6:7], AF.Exp,
                                     scale=-0.5)

            def oln_stage_nmr(st):
                nc.vector.scalar_tensor_tensor(
                    st[:, 7:8], st[:, 2:3], -1.0, st[:, 5:6],
                    op0=ALU.mult, op1=ALU.mult)

            def oln_stage_norm(x, xc, st, dve=False):
                if dve:
                    # (x - mu) * rstd as one 2-scalar DVE op (2x_2P mode)
                    nc.vector.tensor_scalar(
                        xc[:], x[:], st[:, 3:4], st[:, 5:6],
                        op0=ALU.add, op1=ALU.mult)
                else:
                    nc.scalar.activation(xc[:], x[:], AF.Identity,
                                         bias=st[:, 7:8], scale=st[:, 5:6])

            def oln_stage_gb(tt, xc):
                if not triv_gb:
                    nc.vector.tensor_tensor(xc[:], xc[:], g_re[:],
                                            op=ALU.mult)
                    nc.vector.tensor_tensor(xc[:], xc[:], b_re[:],
                                            op=ALU.add)
                nc.sync.dma_start(out_d[tt * 128:(tt + 1) * 128, :], xc[:])

            def oln_tile(tt):
                # used under the last attend: keep ACT free for exp by
                # running the heavy Square/normalize passes on DVE
                x, st = oln_stage_oproj(tt)
                xc = oln_stage_sq(x, st, dve=True)
                oln_stage_stats1(st)
                oln_stage_rstd(st)
                oln_stage_norm(x, xc, st, dve=True)
                oln_stage_gb(tt, xc)

            def oln_tail(tts):
                xs, xcs, sts = {}, {}, {}
                for tt in tts:
                    xs[tt], sts[tt] = oln_stage_oproj(tt)
                for tt in tts:
                    xcs[tt] = oln_stage_sq(xs[tt], sts[tt])
                for tt in tts:
                    oln_stage_stats1(sts[tt])
                for tt in tts:
                    oln_stage_rstd(sts[tt])
                for tt in tts:
                    oln_stage_nmr(sts[tt])
                for tt in tts:
                    oln_stage_norm(xs[tt], xcs[tt], sts[tt])
                for tt in tts:
                    oln_stage_gb(tt, xcs[tt])

            # ---- attention for one (head-pair, i-chunk), with fillers.
            # Software-pipelined: S-pairs run TWO j-steps ahead of their
            # exp, and PV/den consume e one exp behind, so neither PE nor
            # ACT ever waits on the other in steady state (psS bufs=2 and
            # the e-ring provide exactly the needed decoupling). ----
            def attend(hp, ic, fillers):
                io = ic * ICS
                h0, h1 = 2 * hp, 2 * hp + 1
                pv = psPV.tile([128, 2 * ICS], F32, tag="pv")

                def s_pair(jc):
                    s = psS.tile([128, 2 * ICS], F32, tag="s")
                    nc.tensor.matmul(
                        s[:, 0:ICS],
                        KT[0:64, hp * TK + jc * 128: hp * TK + (jc + 1) * 128],
                        QT[0:64, hp * TQ + io: hp * TQ + io + ICS],
                        start=True, stop=True, tile_position=(0, 0))
                    nc.tensor.matmul(
                        s[:, ICS:2 * ICS],
                        KT[64:128, hp * TK + jc * 128: hp * TK + (jc + 1) * 128],
                        QT[64:128, hp * TQ + io: hp * TQ + io + ICS],
                        start=True, stop=True, tile_position=(64, 0))
                    return s

                def exp_op(s):
                    e = expp.tile([128, 2 * ICS], BF16, tag="e")
                    # wq is sent UNSCALED (fp8 can't hold wq/8 — subnormal
                    # territory); 1/sqrt(DH) rides ACT's free affine scale.
                    nc.scalar.activation(e[:], s[:], AF.Exp, scale=SCALE_QK,
                                         bias=mb[:, jc_of[id(s)]:jc_of[id(s)] + 1])
                    return e

                def pv_den(jc, e):
                    st_, sp = (jc == 0), (jc == KC - 1)
                    nc.tensor.matmul(
                        pv[0:64, 0:ICS],
                        V[:, jc * F + h0 * DH: jc * F + (h0 + 1) * DH],
                        e[:, 0:ICS], start=st_, stop=sp,
                        tile_position=(0, 0), skip_group_check=True)
                    nc.tensor.matmul(
                        pv[64:128, 0:ICS],
                        V[:, jc * F + h1 * DH: jc * F + (h1 + 1) * DH],
                        e[:, ICS:2 * ICS], start=st_, stop=sp,
                        tile_position=(0, 64), skip_group_check=True)
                    nc.tensor.matmul(
                        pv[0:64, ICS:2 * ICS], ones[:, 0:64],
                        e[:, 0:ICS], start=st_, stop=sp,
                        tile_position=(0, 0), skip_group_check=True)
                    nc.tensor.matmul(
                        pv[64:128, ICS:2 * ICS], ones[:, 0:64],
                        e[:, ICS:2 * ICS], start=st_, stop=sp,
                        tile_position=(0, 64), skip_group_check=True)

                jc_of = {}
                s0 = s_pair(0); jc_of[id(s0)] = 0
                e_prev = exp_op(s0)
                s_next = s_pair(1); jc_of[id(s_next)] = 1
                for jc in range(KC):
                    if jc + 1 < KC:
                        e_next = exp_op(s_next)
                    if jc + 2 < KC:
                        s_next = s_pair(jc + 2); jc_of[id(s_next)] = jc + 2
                    if fillers:
                        fillers.popleft()()
                    pv_den(jc, e_prev)
                    if jc + 1 < KC:
                        e_prev = e_next
                if fillers:
                    fillers.popleft()()
                # normalize straight out of PSUM: recip(den) then
                # AVT = pvP * rec -- the two reads free psPV for the
                # next attend; no evacuation copy.
                rec = evac.tile([128, ICS], F32, tag="rec")
                nc.vector.reciprocal_approx_fast(rec[:], pv[:, ICS:2 * ICS])
                nc.vector.tensor_tensor(
                    AVT[:, hp, io:io + ICS],
                    pv[:, 0:ICS], rec[:], op=ALU.mult)

            # ---- build per-iteration filler lists ----
            from collections import deque
            k_chunks = _chunks(TK, 512)
            # V chains: group A covers head-pairs 0-3 (f 0:512), B covers
            # 4-7. A(jc0) + Q0/K0 run in the prologue; A(jc>=1) fill iter 0
            # just-in-time (filler jc-1 lands between exp(jc-1) and PV(jc-1),
            # i.e. before PV(jc) needs V[jc]).  B spreads over iters 1-3.
            fillers = {hp: deque() for hp in range(8)}
            for jc in range(1, KC):
                fillers[0].append(lambda jc=jc: v_chain(jc, 0, 512))
            vb = deque(range(KC))
            for hp in range(1, 4):
                for _ in range(3):
                    if vb:
                        jc = vb.popleft()
                        fillers[hp].append(
                            lambda jc=jc: v_chain(jc, 512, 512))
            for hp in range(7):
                fillers[hp].append(lambda hp=hp: q_chain(hp + 1, 0))
                fillers[hp].append(lambda hp=hp: q_chain(hp + 1, 1))
                for t0, tn in k_chunks:
                    fillers[hp].append(
                        lambda hp=hp, t0=t0, tn=tn: k_chain(hp + 1, t0, tn))

            # ---- prologue: minimal deps for attend(0, ic0), emitted in
            # DMA-arrival order so the PE FIFO is never head-blocked ----
            for t0, tn in k_chunks:
                k_chain(0, t0, tn)
            v_chain(0, 0, 512)
            q_chain(0, 0)
            fillers[0].appendleft(lambda: q_chain(0, 1))

            # ---- main pipeline ----
            for hp in range(8):
                attend(hp, 0, fillers[hp])
                if hp == 6:
                    # wo reuses wq's ring slot; its DMA is emitted after the
                    # last wq consumer (Q[7] chains, queued in fillers[6])
                    # so the ring dependency resolves without deadlock.
                    woT = wts.tile([128, FC, D], FP8, tag="wqo")
                    nc.sync.dma_start(woT[:], woT_d[:])
                    if not triv_gb:
                        nc.sync.dma_start(g_re[:], g_d[:])
                        nc.sync.dma_start(b_re[:], b_d[:])
                if hp == 7:
                    ic1_fillers = deque(
                        [lambda tt=tt: oln_tile(tt) for tt in range(4)])
                    attend(hp, 1, ic1_fillers)
                    while ic1_fillers:
                        ic1_fillers.popleft()()
                else:
                    attend(hp, 1, fillers[hp])
                while fillers[hp]:
                    fillers[hp].popleft()()

            # ---- tail: second query-half epilogues, stage-major ----
            oln_tail(list(range(4, TT)))

    nc.compile()
    return nc


def choose_tk(attn_mask):
    """Compacted key count: max unmasked count over batches, ceil to 128."""
    m = np.asarray(attn_mask)
    counts = (~m).sum(axis=0)
    tk = int(((int(counts.max()) + 127) // 128) * 128)
    return max(tk, 128)


def host_prep_core(c, tk, h, attn_mask, wq, wkv, wo, ln_g, ln_b, NH=16, DH=64):
    """Build the per-core input map (numpy) for core c."""
    T, B, D = h.shape
    F = NH * DH
    TQ = T // 2
    KC = tk // 128
    b, qh = c // 2, c % 2
    bf = ml_dtypes.bfloat16
    f8 = ml_dtypes.float8_e4m3fn
    hb = np.roll(np.asarray(h[:, b, :], dtype=np.float32), -qh * TQ, axis=0)
    maskb = np.roll(np.asarray(attn_mask[:, b]), -qh * TQ)
    idx = np.nonzero(~maskb)[0]
    nk = idx.shape[0]
    assert nk <= tk
    idxp = np.concatenate([idx, np.zeros(tk - nk, np.int64)])
    DC = D // 128
    ICS = 512

    def swiz(a):  # [D, X] -> [128, DC, X] partition-major
        return np.ascontiguousarray(
            a.reshape(DC, 128, a.shape[1]).transpose(1, 0, 2))

    hbT = hb.T.astype(f8)                                   # [D, T]
    m = {}
    hTq = swiz(hbT[:, :TQ])
    m["hTqh"], m["hTqr"] = (np.ascontiguousarray(hTq[:, :, :ICS]),
                            np.ascontiguousarray(hTq[:, :, ICS:]))
    m["hTk"] = swiz(hbT[:, idxp])
    m["hq"] = np.ascontiguousarray(hb[:TQ])                 # [TQ, D] f32
    # wq goes UNSCALED (wq/8 would be fp8-subnormal); the kernel applies
    # 1/sqrt(DH) inside the Exp activation instead.
    wq8 = swiz(wq.T.astype(f8))
    m["wqh"], m["wqr"] = (np.ascontiguousarray(wq8[:, :, :128]),
                          np.ascontiguousarray(wq8[:, :, 128:]))
    wk8 = swiz(wkv[:F].T.astype(f8))
    m["wkh"], m["wkr"] = (np.ascontiguousarray(wk8[:, :, :128]),
                          np.ascontiguousarray(wk8[:, :, 128:]))
    wv8 = swiz(wkv[F:].T.astype(f8))
    m["wvh"], m["wvr"] = (np.ascontiguousarray(wv8[:, :, :512]),
                          np.ascontiguousarray(wv8[:, :, 512:]))
    m["woT"] = swiz(wo.T.astype(f8))
    mbias = np.full(tk, NEG_BIG, np.float32)
    mbias[:nk] = 0.0
    m["maskbias"] = np.ascontiguousarray(mbias.reshape(KC, 128).T)
    m["g_rep"] = np.ascontiguousarray(
        np.broadcast_to(np.asarray(ln_g, np.float32), (128, D)))
    m["b_rep"] = np.ascontiguousarray(
        np.broadcast_to(np.asarray(ln_b, np.float32), (128, D)))
    return m

# ======================================================================
# Host-side runner: shard, compile (cached), execute on 8 cores, gather.
# ======================================================================
_NC_CACHE = {}
LAST_RESULT = None  # BassKernelResults of the most recent kernel() call


def _get_nc(T, TQ, TK, D, NH, DH, triv_gb):
    key = (T, TQ, TK, D, NH, DH, triv_gb)
    if key not in _NC_CACHE:
        _NC_CACHE[key] = build_nc(T, TQ, TK, D, NH, DH, n_cores=8,
                                  debug=False, triv_gb=triv_gb)
    return _NC_CACHE[key]


def kernel(h, attn_mask, wq, wkv, wo, ln_g, ln_b):
    """Full-input MultiHeadAttn forward on 8 NeuronCores.

    h: [T, B, D] f32; attn_mask: [T, B] bool (True = masked key);
    wq: [F, D]; wkv: [2F, D]; wo: [D, F]; ln_g/ln_b: [D].
    Returns [T, B, D] f32 = layer_norm(h + attn(h)).
    """
    from concourse.bass_utils import run_bass_kernel_spmd
    global LAST_RESULT

    h = np.asarray(h)
    attn_mask = np.asarray(attn_mask)
    wq = np.asarray(wq, np.float32)
    wkv = np.asarray(wkv, np.float32)
    wo = np.asarray(wo, np.float32)
    ln_g = np.asarray(ln_g, np.float32)
    ln_b = np.asarray(ln_b, np.float32)

    T, B, D = h.shape
    NH = 16
    DH = wq.shape[0] // NH
    assert 2 * B == 8, "sharding assumes batch 4 over 8 cores"
    TQ = T // 2
    TK = min(choose_tk(attn_mask), T)

    triv_gb = bool(np.all(ln_g == 1.0) and np.all(ln_b == 0.0))
    nc = _get_nc(T, TQ, TK, D, NH, DH, triv_gb)
    in_maps = [host_prep_core(c, TK, h, attn_mask, wq, wkv, wo, ln_g, ln_b,
                              NH=NH, DH=DH) for c in range(8)]
    res = run_bass_kernel_spmd(nc, in_maps, core_ids=list(range(8)))
    LAST_RESULT = res

    out = np.empty((T, B, D), np.float32)
    for c in range(8):
        b, qh = c // 2, c % 2
        out[qh * TQ:(qh + 1) * TQ, b, :] = res.results[c]["out"]
    return out
